# revision 1
# baseline (speedup 1.0000x reference)
"""Instant-NGP HashGrid voxel kernel for 8 Trainium2 NeuronCores (Bass).

Data-parallel over points: each core processes N/8 = 32768 points, hash
table + MLP weights replicated. Per batch: DVE computes all 128 corner
indices per point (hash via exact split-multiply int32 ops), PE transposes
the index tile into the column-wrapped order the SWDGE indirect-DMA
consumes, 128 indirect gathers (one per dest partition) fetch 8-byte
entries, DVE does the trilinear lerp tree fused across levels, PE runs the
32->64->1 MLP with relu/sigmoid on ScalarE. Raw-Block manual semaphores
(Tile's attached multi-waits break walrus codegen here).
"""
import sys
sys.path.insert(0, "/opt/trn_rl_repo")
import numpy as np

L = 16
F = 2
T = 1 << 19
MASKC = T - 1
BASE = 16
SCALE = 1.447269237440378
N_PTS = 64 * 64 * 64
P2 = 2654435761
P3 = 805459861

RES = np.floor(BASE * SCALE ** np.arange(L) + 1e-6).astype(np.int64)
DENSEL = (RES + 1) ** 3 <= T
N_DENSE = int(DENSEL.sum())
N_HASH = L - N_DENSE

P2p, P3p = P2 & MASKC, P3 & MASKC
P2h, P2l = P2p >> 7, P2p & 127
P3h, P3l = P3p >> 7, P3p & 127

N_CORES = 8
PTS_PER_CORE = N_PTS // N_CORES
N_BATCHES = 8
B_PER_PART = 32


def build_nc(NB=N_BATCHES, b=B_PER_PART, debug=False):
    import concourse.bass as bass
    import concourse.mybir as mybir

    fp32 = mybir.dt.float32
    i32 = mybir.dt.int32
    AOT = mybir.AluOpType
    AFT = mybir.ActivationFunctionType
    Bpts = 128 * b
    W = 8 * L * b            # idx cols per partition
    w = b                    # cols per gather window (W/128)
    nb = N_DENSE * b
    nh = N_HASH * b
    Lb = L * b
    Lb2 = L * b * F
    CH = min(512, Bpts)
    n_ch = Bpts // CH
    nc = bass.Bass()

    pts_in = nc.declare_dram_parameter("pts", [128, NB * 3 * Lb], fp32, isOutput=False)
    tab = nc.declare_dram_parameter("tab", [L * T * F], fp32, isOutput=False)
    w1t_in = nc.declare_dram_parameter("w1t", [32, 64], fp32, isOutput=False)
    w2t_in = nc.declare_dram_parameter("w2t", [64, 1], fp32, isOutput=False)
    cfw_in = nc.declare_dram_parameter("cfw", [128, 11 * Lb], fp32, isOutput=False)
    ciw_in = nc.declare_dram_parameter("ciw", [128, 7 * Lb], i32, isOutput=False)
    id_in = nc.declare_dram_parameter("idm", [128, 128], fp32, isOutput=False)
    out = nc.declare_dram_parameter("out", [NB, Bpts], fp32, isOutput=True)
    if debug:
        dIDX = nc.declare_dram_parameter("dIDX", [128, W], fp32, isOutput=True)
        dO = nc.declare_dram_parameter("dO", [128, W], i32, isOutput=True)
        dG = nc.declare_dram_parameter("dG", [128, W * F], fp32, isOutput=True)
        dE2 = nc.declare_dram_parameter("dE2", [128, Lb2], fp32, isOutput=True)
        dET = nc.declare_dram_parameter("dET", [32, Bpts], fp32, isOutput=True)

    tabv = tab[:].rearrange("(t f) -> t f", f=F)

    ctx = []

    def sb(shape, dt):
        cm = nc.sbuf_tensor(shape, dt)
        t_ = cm.__enter__(); ctx.append(cm); return t_

    def ps(shape, dt):
        cm = nc.psum_tensor(shape, dt)
        t_ = cm.__enter__(); ctx.append(cm); return t_

    ident = sb([128, 128], fp32)
    w1t = sb([32, 64], fp32)
    w2t = sb([64, 1], fp32)
    cfw = sb([128, 11 * Lb], fp32)
    ciw = sb([128, 7 * Lb], i32)
    ptsb = sb([128, 3 * Lb], fp32)
    pos = [sb([128, Lb], fp32) for _ in range(3)]
    ci = [sb([128, Lb], i32) for _ in range(3)]
    c0f = [sb([128, Lb], fp32) for _ in range(3)]
    frF = [sb([128, Lb2], fp32) for _ in range(3)]
    x1h = sb([128, Lb], i32)
    yP0 = sb([128, nh], i32); yP1 = sb([128, nh], i32)
    zP0 = sb([128, nh], i32); zP1 = sb([128, nh], i32)
    tmpi = sb([128, nh], i32)
    hyz = {k: sb([128, nh], i32) for k in range(4)}
    hidx = sb([128, nh], i32)
    dbase = sb([128, nb], fp32)
    dtmp = sb([128, nb], fp32)
    IDX = sb([128, W], fp32)
    O = sb([128, W], i32)
    G = sb([128, W * F], fp32)
    tmpf = sb([128, Lb2], fp32)
    encl = sb([128, Lb2], fp32)      # (l i f)
    enc2 = sb([128, Lb2], fp32)      # (i l f)
    encT = sb([32, Bpts], fp32)
    hsb = [sb([64, CH], fp32) for _ in range(2)]
    outb = sb([1, Bpts], fp32)
    pT = [ps([128, 128], fp32) for _ in range(2)]
    pE = [ps([32, 128], fp32) for _ in range(2)]
    hps = [ps([64, CH], fp32) for _ in range(2)]
    ops = [ps([1, CH], fp32) for _ in range(2)]

    sd_cm = nc.semaphore(); sd = sd_cm.__enter__(); ctx.append(sd_cm)
    sg_cm = nc.semaphore(); sg = sg_cm.__enter__(); ctx.append(sg_cm)
    sv_cm = nc.semaphore(); sv = sv_cm.__enter__(); ctx.append(sv_cm)
    st_cm = nc.semaphore(); st = st_cm.__enter__(); ctx.append(st_cm)
    sa_cm = nc.semaphore(); sa = sa_cm.__enter__(); ctx.append(sa_cm)

    NCONST = 7          # const DMAs
    STB = b + b + 2 * n_ch        # tensor instrs per batch
    SVB = 1 + b + 1 + b           # vector sem incs per batch
    SAB = 2 * n_ch
    Or = O[:].rearrange("p (j k) -> p k j", k=w)
    eTr = encT[:].rearrange("q (P m) -> q m P", m=b)

    def cslice(tile_, slot, hash_only=False, dense_only=False):
        s = slot * Lb
        if hash_only:
            return tile_[:, s + nb: s + Lb]
        if dense_only:
            return tile_[:, s: s + nb]
        return tile_[:, s: s + Lb]

    blk_cm = nc.Block(); block = blk_cm.__enter__(); ctx.append(blk_cm)

    @block.sync
    def _(sy):
        sy.dma_start(ident[:], id_in[:]).then_inc(sd, 16)
        sy.dma_start(w1t[:], w1t_in[:]).then_inc(sd, 16)
        sy.dma_start(w2t[:], w2t_in[:]).then_inc(sd, 16)
        sy.dma_start(cfw[:], cfw_in[:]).then_inc(sd, 16)
        sy.dma_start(ciw[:], ciw_in[:]).then_inc(sd, 16)
        sy.dma_start(ptsb[:], pts_in[:, 0:3 * Lb]).then_inc(sd, 16)
        for t in range(NB):
            # wait batch t fully written by scalar, then ship out + next pts
            sy.wait_ge(sa, SAB * (t + 1))
            sy.dma_start(out[t:t + 1, :], outb[:]).then_inc(sd, 16)
            if t + 1 < NB:
                sy.dma_start(
                    ptsb[:], pts_in[:, (t + 1) * 3 * Lb:(t + 2) * 3 * Lb]
                ).then_inc(sd, 16)
        if debug:
            sy.dma_start(dIDX[:], IDX[:]).then_inc(sd, 16)
            sy.dma_start(dO[:], O[:]).then_inc(sd, 16)
            sy.dma_start(dG[:], G[:]).then_inc(sd, 16)
            sy.dma_start(dE2[:], enc2[:]).then_inc(sd, 16)
            sy.dma_start(dET[:], encT[:]).then_inc(sd, 16)
            sy.wait_ge(sd, 16 * (6 + 2 * NB - 2) + 80)

    @block.vector
    def _(v):
        for t in range(NB):
            # pts batch ready (NCONST-1 consts + t-th ptsb; out DMAs interleave)
            v.wait_ge(sd, 16 * (6 + 2 * t))
            if t > 0:
                v.wait_ge(st, STB * t)      # tensor done reading IDX/enc/encT
            # ---- floors / fracs ----
            for d in range(3):
                pd = ptsb[:, d * Lb:(d + 1) * Lb]
                v.tensor_tensor(out=pos[d][:], in0=pd, in1=cslice(cfw, 0),
                                op=AOT.mult)
                v.tensor_scalar(out=pos[d][:], in0=pos[d][:], scalar1=-0.5,
                                scalar2=None, op0=AOT.add)
                v.tensor_copy(out=ci[d][:], in_=pos[d][:])
                v.tensor_copy(out=c0f[d][:], in_=ci[d][:])
                # frac = (pos-0.5 - c0f) + 0.5 stored duplicated over feats
                v.tensor_tensor(out=pos[d][:], in0=pos[d][:], in1=c0f[d][:],
                                op=AOT.subtract)
                v.tensor_scalar(out=pos[d][:], in0=pos[d][:], scalar1=0.5,
                                scalar2=None, op0=AOT.add)
                fv = frF[d][:].rearrange("p (x e) -> p x e", e=F)
                v.tensor_copy(out=fv[:, :, 0], in_=pos[d][:])
                v.tensor_copy(out=fv[:, :, 1], in_=pos[d][:])
            # ---- hash products ----
            for (dst, srcci, hi, lo) in ((yP0, ci[1], 0, 1), (zP0, ci[2], 2, 3)):
                v.tensor_tensor(out=dst[:], in0=srcci[:, nb:Lb],
                                in1=cslice(ciw, hi, hash_only=True), op=AOT.mult)
                v.tensor_scalar(out=dst[:], in0=dst[:], scalar1=7, scalar2=None,
                                op0=AOT.logical_shift_left)
                v.tensor_tensor(out=tmpi[:], in0=srcci[:, nb:Lb],
                                in1=cslice(ciw, lo, hash_only=True), op=AOT.mult)
                v.tensor_tensor(out=dst[:], in0=dst[:], in1=tmpi[:], op=AOT.add)
            v.tensor_tensor(out=yP1[:], in0=yP0[:],
                            in1=cslice(ciw, 4, hash_only=True), op=AOT.add)
            v.tensor_tensor(out=zP1[:], in0=zP0[:],
                            in1=cslice(ciw, 5, hash_only=True), op=AOT.add)
            for dy, yy in ((0, yP0), (1, yP1)):
                for dz, zz in ((0, zP0), (1, zP1)):
                    v.tensor_tensor(out=hyz[dy * 2 + dz][:], in0=yy[:],
                                    in1=zz[:], op=AOT.bitwise_xor)
            v.tensor_scalar(out=x1h[:], in0=ci[0][:], scalar1=1, scalar2=None,
                            op0=AOT.add)
            for c in range(8):
                dx, dy, dz = (c >> 2) & 1, (c >> 1) & 1, c & 1
                xx = x1h if dx else ci[0]
                v.tensor_tensor(out=hidx[:], in0=xx[:, nb:Lb],
                                in1=hyz[dy * 2 + dz][:], op=AOT.bitwise_xor)
                v.tensor_scalar(out=hidx[:], in0=hidx[:], scalar1=MASKC,
                                scalar2=None, op0=AOT.bitwise_and)
                v.tensor_tensor(out=hidx[:], in0=hidx[:],
                                in1=cslice(ciw, 6, hash_only=True), op=AOT.add)
                v.tensor_copy(out=IDX[:, c * Lb + nb:(c + 1) * Lb], in_=hidx[:])
            # ---- dense indices (float, exact) ----
            v.tensor_tensor(out=dbase[:], in0=c0f[1][:, 0:nb],
                            in1=cslice(cfw, 1, dense_only=True), op=AOT.mult)
            v.tensor_tensor(out=dbase[:], in0=dbase[:], in1=c0f[0][:, 0:nb],
                            op=AOT.add)
            v.tensor_tensor(out=dtmp[:], in0=c0f[2][:, 0:nb],
                            in1=cslice(cfw, 2, dense_only=True), op=AOT.mult)
            v.tensor_tensor(out=dbase[:], in0=dbase[:], in1=dtmp[:], op=AOT.add)
            for c in range(8):
                v.tensor_tensor(out=IDX[:, c * Lb:c * Lb + nb], in0=dbase[:],
                                in1=cslice(cfw, 3 + c, dense_only=True),
                                op=AOT.add)

            v.tensor_copy(out=tmpi[:, 0:1], in_=tmpi[:, 0:1]).then_inc(sv, 1)
            # ---- copy PE-transposed IDX blocks into O ----
            if t > 0:
                v.wait_ge(sg, 2048 * t)      # gathers of prev batch done (WAR O)
            for k in range(b):
                v.wait_ge(st, STB * t + k + 1)
                v.tensor_copy(out=Or[:, k, :], in_=pT[k % 2][:]).then_inc(sv, 1)
            # ---- wait gathers, lerp ----
            v.wait_ge(sg, 2048 * (t + 1))

            def gc(c):
                return G[:, c * Lb2:(c + 1) * Lb2]

            for c in (0, 2, 4, 6):
                v.tensor_tensor(out=tmpf[:], in0=gc(c + 1), in1=gc(c),
                                op=AOT.subtract)
                v.tensor_tensor(out=tmpf[:], in0=tmpf[:], in1=frF[2][:],
                                op=AOT.mult)
                v.tensor_tensor(out=gc(c), in0=gc(c), in1=tmpf[:], op=AOT.add)
            for c in (0, 4):
                v.tensor_tensor(out=tmpf[:], in0=gc(c + 2), in1=gc(c),
                                op=AOT.subtract)
                v.tensor_tensor(out=tmpf[:], in0=tmpf[:], in1=frF[1][:],
                                op=AOT.mult)
                v.tensor_tensor(out=gc(c), in0=gc(c), in1=tmpf[:], op=AOT.add)
            v.tensor_tensor(out=tmpf[:], in0=gc(4), in1=gc(0), op=AOT.subtract)
            v.tensor_tensor(out=tmpf[:], in0=tmpf[:], in1=frF[0][:],
                            op=AOT.mult)
            v.tensor_tensor(out=encl[:], in0=gc(0), in1=tmpf[:], op=AOT.add)
            # reorder (l i f) -> (i l f): one strided copy per level
            for l in range(L):
                src = encl[:, l * b * F:(l + 1) * b * F].rearrange(
                    "p (i e) -> p i e", e=F)
                dst = enc2[:].rearrange("p (i l e) -> p i l e", l=L, e=F)[:, :, l, :]
                v.tensor_copy(out=dst, in_=src)
            v.tensor_copy(out=tmpi[:, 0:1], in_=tmpi[:, 0:1]).then_inc(sv, 1)
            # ---- copy PE-transposed enc blocks into encT ----
            for i in range(b):
                v.wait_ge(st, STB * t + b + i + 1)
                v.tensor_copy(out=eTr[:, i, :], in_=pE[i % 2][:]).then_inc(sv, 1)

    @block.tensor
    def _(te):
        te.wait_ge(sd, 16)       # identity loaded
        for t in range(NB):
            te.wait_ge(sv, SVB * t + 1)            # IDX ready
            for k in range(b):
                if k >= 2:
                    te.wait_ge(sv, SVB * t + 1 + (k - 1))   # bank freed
                te.transpose(pT[k % 2][:], IDX[:, 128 * k:128 * (k + 1)],
                             ident[:]).then_inc(st, 1)
            te.wait_ge(sv, SVB * t + b + 2)        # enc2 ready
            for i in range(b):
                if i >= 2:
                    te.wait_ge(sv, SVB * t + b + 2 + (i - 1))
                te.transpose(pE[i % 2][:], enc2[:, i * 32:(i + 1) * 32],
                             ident[:]).then_inc(st, 1)
            te.wait_ge(sv, SVB * (t + 1))          # encT complete
            for ch in range(n_ch):
                if ch >= 2:
                    te.wait_ge(sa, SAB * t + 2 * (ch - 2) + 1)
                te.matmul(hps[ch % 2][:], w1t[:],
                          encT[:, ch * CH:(ch + 1) * CH],
                          start=True, stop=True).then_inc(st, 1)
                te.wait_ge(sa, SAB * t + 2 * ch + 1)
                te.matmul(ops[ch % 2][:], w2t[:], hsb[ch % 2][:],
                          start=True, stop=True).then_inc(st, 1)

    @block.gpsimd
    def _(g):
        for t in range(NB):
            g.wait_ge(sv, SVB * t + 1 + b)         # O complete
            if t > 0:
                g.wait_ge(sv, SVB * (t - 1) + b + 2)  # lerps of t-1 read G
            for j in range(128):
                g.indirect_dma_start(
                    out=G[j:j + 1, :].rearrange("p (k e) -> p k e", e=F),
                    out_offset=None,
                    in_=tabv,
                    in_offset=bass.IndirectOffsetOnAxis(
                        ap=O[:, j * w:(j + 1) * w], axis=0),
                ).then_inc(sg, 16)

    @block.scalar
    def _(ac):
        for t in range(NB):
            if t > 0:
                ac.wait_ge(sd, 16 * (6 + 2 * t) - 16)  # outb shipped (WAR)
            for ch in range(n_ch):
                ac.wait_ge(st, STB * t + 2 * b + 2 * ch + 1)
                ac.activation(hsb[ch % 2][:], hps[ch % 2][:],
                              AFT.Relu).then_inc(sa, 1)
                ac.wait_ge(st, STB * t + 2 * b + 2 * ch + 2)
                ac.activation(outb[:, ch * CH:(ch + 1) * CH], ops[ch % 2][:],
                              AFT.Sigmoid).then_inc(sa, 1)

    for cm in reversed(ctx):
        cm.__exit__(None, None, None)
    return nc


# ---------------- host side ----------------

class _Runner:
    def __init__(self, nc, n_cores):
        import jax
        import numpy as _np
        from jax.sharding import Mesh, PartitionSpec
        from jax.experimental.shard_map import shard_map
        import concourse.mybir as mybir
        from concourse.bass2jax import (
            install_neuronx_cc_hook, _bass_exec_p, partition_id_tensor)
        install_neuronx_cc_hook()
        self.n_cores = n_cores
        pname = nc.partition_id_tensor.name if nc.partition_id_tensor else None
        in_names, out_names, out_avals, zero_outs = [], [], [], []
        for alloc in nc.m.functions[0].allocations:
            if not isinstance(alloc, mybir.MemoryLocationSet):
                continue
            name = alloc.memorylocations[0].name
            if alloc.kind == "ExternalInput":
                if name != pname:
                    in_names.append(name)
            elif alloc.kind == "ExternalOutput":
                shape = tuple(alloc.tensor_shape)
                dtype = mybir.dt.np(alloc.dtype)
                out_names.append(name)
                out_avals.append(jax.core.ShapedArray(shape, dtype))
                zero_outs.append(_np.zeros(shape, dtype))
        self.in_names, self.out_names = in_names, out_names
        self.out_avals, self.zero_outs = out_avals, zero_outs
        n_params, n_outs = len(in_names), len(out_names)
        all_in = in_names + out_names + ([pname] if pname else [])

        def _body(*args):
            operands = list(args)
            if pname is not None:
                operands.append(partition_id_tensor())
            return tuple(_bass_exec_p.bind(
                *operands, out_avals=tuple(out_avals), in_names=tuple(all_in),
                out_names=tuple(out_names), lowering_input_output_aliases=(),
                sim_require_finite=True, sim_require_nnan=True, nc=nc))

        self.n_params, self.n_outs = n_params, n_outs
        donate = tuple(range(n_params, n_params + n_outs))
        devices = jax.devices()[:n_cores]
        mesh = Mesh(_np.asarray(devices), ("core",))
        specs = (PartitionSpec("core"),)
        self.fn = jax.jit(
            shard_map(_body, mesh=mesh, in_specs=specs * (n_params + n_outs),
                      out_specs=specs * n_outs, check_rep=False),
            donate_argnums=donate, keep_unused=True)

    def __call__(self, in_maps):
        import numpy as _np
        n = self.n_cores
        per_core = [[_np.asarray(m[nm]) for nm in self.in_names]
                    for m in in_maps]
        concat_in = [_np.concatenate([per_core[c][i] for c in range(n)], axis=0)
                     for i in range(self.n_params)]
        concat_zeros = [_np.zeros((n * z.shape[0], *z.shape[1:]), z.dtype)
                        for z in self.zero_outs]
        outs = self.fn(*concat_in, *concat_zeros)
        return [
            {nm: _np.asarray(outs[i]).reshape(n, *self.out_avals[i].shape)[c]
             for i, nm in enumerate(self.out_names)}
            for c in range(n)
        ]


_RUNNERS = {}


def _get_runner(NB, b):
    key = (NB, b)
    if key not in _RUNNERS:
        _RUNNERS[key] = _Runner(build_nc(NB, b), N_CORES)
    return _RUNNERS[key]


def _consts(b):
    Lb = L * b
    cfw = np.zeros((128, 11 * Lb), np.float32)
    ciw = np.zeros((128, 7 * Lb), np.int32)
    r1 = np.where(DENSEL, RES + 1, 0)

    def setf(slot, vals):
        cfw[:, slot * Lb:(slot + 1) * Lb] = np.repeat(
            np.asarray(vals, np.float64), b)[None, :]

    def seti(slot, vals):
        ciw[:, slot * Lb:(slot + 1) * Lb] = np.repeat(
            np.asarray(vals, np.int64), b).astype(np.int32)[None, :]

    setf(0, RES)
    setf(1, r1)
    setf(2, r1 * r1)
    for c in range(8):
        dx, dy, dz = (c >> 2) & 1, (c >> 1) & 1, c & 1
        setf(3 + c, np.where(DENSEL,
                             dx + r1 * dy + r1 * r1 * dz + np.arange(L) * T, 0))
    z = np.zeros(L, np.int64)

    def hv(val):
        a = z.copy(); a[N_DENSE:] = val; return a

    seti(0, hv(P2h)); seti(1, hv(P2l)); seti(2, hv(P3h)); seti(3, hv(P3l))
    seti(4, hv(P2p)); seti(5, hv(P3p))
    seti(6, np.arange(L) * T)
    return cfw, ciw


def _prep_core_inputs(points_core, tabflat, w1t, w2t, cfw, ciw, NB, b):
    # pts layout: [128, NB, 3, L, b]; point (t, p, i) coord d replicated /level
    p4 = points_core.reshape(NB, 128, b, 3).transpose(1, 0, 3, 2)  # p t d i
    p5 = np.repeat(p4[:, :, :, None, :], L, axis=3)                # p t d l i
    pts = np.ascontiguousarray(p5, np.float32).reshape(128, NB * 3 * L * b)
    return {"pts": pts, "tab": tabflat, "w1t": w1t, "w2t": w2t,
            "cfw": cfw, "ciw": ciw, "idm": np.eye(128, dtype=np.float32)}


def kernel(points, table, w1, w2):
    points = np.asarray(points, np.float32)
    table = np.asarray(table, np.float32)
    tabflat = np.ascontiguousarray(table.reshape(L * T * F))
    w1t = np.ascontiguousarray(np.asarray(w1, np.float32).T)
    w2t = np.ascontiguousarray(np.asarray(w2, np.float32).T)
    NB, b = N_BATCHES, B_PER_PART
    cfw, ciw = _consts(b)
    runner = _get_runner(NB, b)
    in_maps = [
        _prep_core_inputs(points[c * PTS_PER_CORE:(c + 1) * PTS_PER_CORE],
                          tabflat, w1t, w2t, cfw, ciw, NB, b)
        for c in range(N_CORES)
    ]
    res = runner(in_maps)
    outs = [res[c]["out"].reshape(-1) for c in range(N_CORES)]
    return np.concatenate(outs).reshape(1, 64, 64, 64).astype(np.float32)



# revision 7
# speedup vs baseline: 1.1927x; 1.1927x over previous
"""Instant-NGP HashGrid voxel kernel for 8 Trainium2 NeuronCores (Bass).

Data-parallel over points: each core processes N/8 = 32768 points, hash
table + MLP weights replicated. Per batch: DVE computes all 128 corner
indices per point (hash via exact split-multiply int32 ops), PE transposes
the index tile into the column-wrapped order the SWDGE indirect-DMA
consumes, 128 indirect gathers (one per dest partition) fetch 8-byte
entries, DVE does the trilinear lerp tree fused across levels, PE runs the
32->64->1 MLP with relu/sigmoid on ScalarE. Raw-Block manual semaphores
(Tile's attached multi-waits break walrus codegen here).
"""
import sys
sys.path.insert(0, "/opt/trn_rl_repo")
import numpy as np

L = 16
F = 2
T = 1 << 19
MASKC = T - 1
BASE = 16
SCALE = 1.447269237440378
N_PTS = 64 * 64 * 64
P2 = 2654435761
P3 = 805459861

RES = np.floor(BASE * SCALE ** np.arange(L) + 1e-6).astype(np.int64)
DENSEL = (RES + 1) ** 3 <= T
N_DENSE = int(DENSEL.sum())
N_HASH = L - N_DENSE

P2p, P3p = P2 & MASKC, P3 & MASKC
P2h, P2l = P2p >> 7, P2p & 127
P3h, P3l = P3p >> 7, P3p & 127

N_CORES = 8
PTS_PER_CORE = N_PTS // N_CORES
N_BATCHES = 8
B_PER_PART = 32


def build_nc(NB=N_BATCHES, b=B_PER_PART, debug=False):
    import concourse.bass as bass
    import concourse.mybir as mybir

    fp32 = mybir.dt.float32
    i32 = mybir.dt.int32
    AOT = mybir.AluOpType
    AFT = mybir.ActivationFunctionType
    Bpts = 128 * b
    W = 8 * L * b            # idx cols per partition
    w = b                    # cols per gather window (W/128)
    nb = N_DENSE * b
    nh = N_HASH * b
    Lb = L * b
    Lb2 = L * b * F
    CH = min(512, Bpts)
    n_ch = Bpts // CH
    nc = bass.Bass()

    pts_in = nc.declare_dram_parameter("pts", [128, NB * 3 * Lb], fp32, isOutput=False)
    tab = nc.declare_dram_parameter("tab", [L * T * F], fp32, isOutput=False)
    w1t_in = nc.declare_dram_parameter("w1t", [32, 64], fp32, isOutput=False)
    w2t_in = nc.declare_dram_parameter("w2t", [64, 1], fp32, isOutput=False)
    cfw_in = nc.declare_dram_parameter("cfw", [128, 11 * Lb], fp32, isOutput=False)
    ciw_in = nc.declare_dram_parameter("ciw", [128, 7 * Lb], i32, isOutput=False)
    id_in = nc.declare_dram_parameter("idm", [128, 128], fp32, isOutput=False)
    out = nc.declare_dram_parameter("out", [NB, Bpts], fp32, isOutput=True)
    if debug:
        dIDX = nc.declare_dram_parameter("dIDX", [128, W], fp32, isOutput=True)
        dO = nc.declare_dram_parameter("dO", [128, W], i32, isOutput=True)
        dG = nc.declare_dram_parameter("dG", [128, W * F], fp32, isOutput=True)
        dE2 = nc.declare_dram_parameter("dE2", [128, Lb2], fp32, isOutput=True)
        dET = nc.declare_dram_parameter("dET", [32, Bpts], fp32, isOutput=True)

    tabv = tab[:].rearrange("(t f) -> t f", f=F)

    ctx = []

    def sb(shape, dt):
        cm = nc.sbuf_tensor(shape, dt)
        t_ = cm.__enter__(); ctx.append(cm); return t_

    def ps(shape, dt):
        cm = nc.psum_tensor(shape, dt)
        t_ = cm.__enter__(); ctx.append(cm); return t_

    ident = sb([128, 128], fp32)
    w1t = sb([32, 64], fp32)
    w2t = sb([64, 1], fp32)
    cfw = sb([128, 11 * Lb], fp32)
    ciw = sb([128, 7 * Lb], i32)
    ptsb = sb([128, 3 * Lb], fp32)
    pos = [sb([128, Lb], fp32) for _ in range(3)]
    ci = [sb([128, Lb], i32) for _ in range(3)]
    c0f = [sb([128, Lb], fp32) for _ in range(3)]
    frF = [sb([128, Lb2], fp32) for _ in range(3)]
    x1h = sb([128, Lb], i32)
    yP0 = sb([128, nh], i32); yP1 = sb([128, nh], i32)
    zP0 = sb([128, nh], i32); zP1 = sb([128, nh], i32)
    tmpi = sb([128, nh], i32)
    hyz = {k: sb([128, nh], i32) for k in range(4)}
    hidx = sb([128, nh], i32)
    dbase = sb([128, nb], fp32)
    dtmp = sb([128, nb], fp32)
    IDX = sb([128, W], fp32)
    O = sb([128, W], i32)
    G = sb([128, W * F], fp32)
    tmpf = sb([128, Lb2], fp32)
    encl = sb([128, Lb2], fp32)      # (l i f)
    enc2 = sb([128, Lb2], fp32)      # (i l f)
    encT = sb([32, Bpts], fp32)
    hsb = [sb([64, CH], fp32) for _ in range(2)]
    outb = sb([1, Bpts], fp32)
    pT = [ps([128, 128], fp32) for _ in range(2)]
    pE = [ps([32, 128], fp32) for _ in range(2)]
    hps = [ps([64, CH], fp32) for _ in range(2)]
    ops = [ps([1, CH], fp32) for _ in range(2)]

    sd_cm = nc.semaphore(); sd = sd_cm.__enter__(); ctx.append(sd_cm)
    sg_cm = nc.semaphore(); sg = sg_cm.__enter__(); ctx.append(sg_cm)
    sv_cm = nc.semaphore(); sv = sv_cm.__enter__(); ctx.append(sv_cm)
    st_cm = nc.semaphore(); st = st_cm.__enter__(); ctx.append(st_cm)
    sa_cm = nc.semaphore(); sa = sa_cm.__enter__(); ctx.append(sa_cm)

    NCONST = 7          # const DMAs
    GPB = 16 * 128 * (w // 16)    # gather sem increments per batch
    STB = b + b + 2 * n_ch        # tensor instrs per batch
    SVB = 1 + b + 1 + b           # vector sem incs per batch
    SAB = 2 * n_ch
    Or = O[:].rearrange("p (j k) -> p k j", k=w)
    eTr = encT[:].rearrange("q (P m) -> q m P", m=b)

    def cslice(tile_, slot, hash_only=False, dense_only=False):
        s = slot * Lb
        if hash_only:
            return tile_[:, s + nb: s + Lb]
        if dense_only:
            return tile_[:, s: s + nb]
        return tile_[:, s: s + Lb]

    blk_cm = nc.Block(); block = blk_cm.__enter__(); ctx.append(blk_cm)

    @block.sync
    def _(sy):
        sy.dma_start(ident[:], id_in[:]).then_inc(sd, 16)
        sy.dma_start(w1t[:], w1t_in[:]).then_inc(sd, 16)
        sy.dma_start(w2t[:], w2t_in[:]).then_inc(sd, 16)
        sy.dma_start(cfw[:], cfw_in[:]).then_inc(sd, 16)
        sy.dma_start(ciw[:], ciw_in[:]).then_inc(sd, 16)
        sy.dma_start(ptsb[:], pts_in[:, 0:3 * Lb]).then_inc(sd, 16)
        for t in range(NB):
            # wait batch t fully written by scalar, then ship out + next pts
            sy.wait_ge(sa, SAB * (t + 1))
            sy.dma_start(out[t:t + 1, :], outb[:]).then_inc(sd, 16)
            if t + 1 < NB:
                sy.dma_start(
                    ptsb[:], pts_in[:, (t + 1) * 3 * Lb:(t + 2) * 3 * Lb]
                ).then_inc(sd, 16)
        if debug:
            sy.dma_start(dIDX[:], IDX[:]).then_inc(sd, 16)
            sy.dma_start(dO[:], O[:]).then_inc(sd, 16)
            sy.dma_start(dG[:], G[:]).then_inc(sd, 16)
            sy.dma_start(dE2[:], enc2[:]).then_inc(sd, 16)
            sy.dma_start(dET[:], encT[:]).then_inc(sd, 16)
            sy.wait_ge(sd, 16 * (6 + 2 * NB - 2) + 80)

    @block.vector
    def _(v):
        for t in range(NB):
            # pts batch ready (NCONST-1 consts + t-th ptsb; out DMAs interleave)
            v.wait_ge(sd, 16 * (6 + 2 * t))
            if t > 0:
                v.wait_ge(st, STB * t)      # tensor done reading IDX/enc/encT
            # ---- floors / fracs ----
            for d in range(3):
                pd = ptsb[:, d * Lb:(d + 1) * Lb]
                v.tensor_tensor(out=pos[d][:], in0=pd, in1=cslice(cfw, 0),
                                op=AOT.mult)
                v.tensor_scalar(out=pos[d][:], in0=pos[d][:], scalar1=-0.5,
                                scalar2=None, op0=AOT.add)
                v.tensor_copy(out=ci[d][:], in_=pos[d][:])
                v.tensor_copy(out=c0f[d][:], in_=ci[d][:])
                # frac = (pos-0.5 - c0f) + 0.5 stored duplicated over feats
                v.tensor_tensor(out=pos[d][:], in0=pos[d][:], in1=c0f[d][:],
                                op=AOT.subtract)
                v.tensor_scalar(out=pos[d][:], in0=pos[d][:], scalar1=0.5,
                                scalar2=None, op0=AOT.add)
                fv = frF[d][:].rearrange("p (x e) -> p x e", e=F)
                v.tensor_copy(out=fv[:, :, 0], in_=pos[d][:])
                v.tensor_copy(out=fv[:, :, 1], in_=pos[d][:])
            # ---- hash products ----
            for (dst, srcci, hi, lo) in ((yP0, ci[1], 0, 1), (zP0, ci[2], 2, 3)):
                v.tensor_tensor(out=dst[:], in0=srcci[:, nb:Lb],
                                in1=cslice(ciw, hi, hash_only=True), op=AOT.mult)
                v.tensor_scalar(out=dst[:], in0=dst[:], scalar1=7, scalar2=None,
                                op0=AOT.logical_shift_left)
                v.tensor_tensor(out=tmpi[:], in0=srcci[:, nb:Lb],
                                in1=cslice(ciw, lo, hash_only=True), op=AOT.mult)
                v.tensor_tensor(out=dst[:], in0=dst[:], in1=tmpi[:], op=AOT.add)
            v.tensor_tensor(out=yP1[:], in0=yP0[:],
                            in1=cslice(ciw, 4, hash_only=True), op=AOT.add)
            v.tensor_tensor(out=zP1[:], in0=zP0[:],
                            in1=cslice(ciw, 5, hash_only=True), op=AOT.add)
            for dy, yy in ((0, yP0), (1, yP1)):
                for dz, zz in ((0, zP0), (1, zP1)):
                    v.tensor_tensor(out=hyz[dy * 2 + dz][:], in0=yy[:],
                                    in1=zz[:], op=AOT.bitwise_xor)
            v.tensor_scalar(out=x1h[:], in0=ci[0][:], scalar1=1, scalar2=None,
                            op0=AOT.add)
            for c in range(8):
                dx, dy, dz = (c >> 2) & 1, (c >> 1) & 1, c & 1
                xx = x1h if dx else ci[0]
                v.tensor_tensor(out=hidx[:], in0=xx[:, nb:Lb],
                                in1=hyz[dy * 2 + dz][:], op=AOT.bitwise_xor)
                v.tensor_scalar(out=hidx[:], in0=hidx[:], scalar1=MASKC,
                                scalar2=None, op0=AOT.bitwise_and)
                v.tensor_tensor(out=hidx[:], in0=hidx[:],
                                in1=cslice(ciw, 6, hash_only=True), op=AOT.add)
                v.tensor_copy(out=IDX[:, c * Lb + nb:(c + 1) * Lb], in_=hidx[:])
            # ---- dense indices (float, exact) ----
            v.tensor_tensor(out=dbase[:], in0=c0f[1][:, 0:nb],
                            in1=cslice(cfw, 1, dense_only=True), op=AOT.mult)
            v.tensor_tensor(out=dbase[:], in0=dbase[:], in1=c0f[0][:, 0:nb],
                            op=AOT.add)
            v.tensor_tensor(out=dtmp[:], in0=c0f[2][:, 0:nb],
                            in1=cslice(cfw, 2, dense_only=True), op=AOT.mult)
            v.tensor_tensor(out=dbase[:], in0=dbase[:], in1=dtmp[:], op=AOT.add)
            for c in range(8):
                v.tensor_tensor(out=IDX[:, c * Lb:c * Lb + nb], in0=dbase[:],
                                in1=cslice(cfw, 3 + c, dense_only=True),
                                op=AOT.add)

            v.tensor_copy(out=tmpi[:, 0:1], in_=tmpi[:, 0:1]).then_inc(sv, 1)
            # ---- copy PE-transposed IDX blocks into O ----
            if t > 0:
                v.wait_ge(sg, GPB * t)       # gathers of prev batch done (WAR O)
            for k in range(b):
                v.wait_ge(st, STB * t + k + 1)
                v.tensor_copy(out=Or[:, k, :], in_=pT[k % 2][:]).then_inc(sv, 1)
            # ---- wait gathers, lerp ----
            v.wait_ge(sg, GPB * (t + 1))

            def gc(c):
                return G[:, c * Lb2:(c + 1) * Lb2]

            for c in (0, 2, 4, 6):
                v.tensor_tensor(out=tmpf[:], in0=gc(c + 1), in1=gc(c),
                                op=AOT.subtract)
                v.tensor_tensor(out=tmpf[:], in0=tmpf[:], in1=frF[2][:],
                                op=AOT.mult)
                v.tensor_tensor(out=gc(c), in0=gc(c), in1=tmpf[:], op=AOT.add)
            for c in (0, 4):
                v.tensor_tensor(out=tmpf[:], in0=gc(c + 2), in1=gc(c),
                                op=AOT.subtract)
                v.tensor_tensor(out=tmpf[:], in0=tmpf[:], in1=frF[1][:],
                                op=AOT.mult)
                v.tensor_tensor(out=gc(c), in0=gc(c), in1=tmpf[:], op=AOT.add)
            v.tensor_tensor(out=tmpf[:], in0=gc(4), in1=gc(0), op=AOT.subtract)
            v.tensor_tensor(out=tmpf[:], in0=tmpf[:], in1=frF[0][:],
                            op=AOT.mult)
            v.tensor_tensor(out=encl[:], in0=gc(0), in1=tmpf[:], op=AOT.add)
            # reorder (l i f) -> (i l f): one strided copy per level
            for l in range(L):
                src = encl[:, l * b * F:(l + 1) * b * F].rearrange(
                    "p (i e) -> p i e", e=F)
                dst = enc2[:].rearrange("p (i l e) -> p i l e", l=L, e=F)[:, :, l, :]
                v.tensor_copy(out=dst, in_=src)
            v.tensor_copy(out=tmpi[:, 0:1], in_=tmpi[:, 0:1]).then_inc(sv, 1)
            # ---- copy PE-transposed enc blocks into encT ----
            for i in range(b):
                v.wait_ge(st, STB * t + b + i + 1)
                v.tensor_copy(out=eTr[:, i, :], in_=pE[i % 2][:]).then_inc(sv, 1)

    @block.tensor
    def _(te):
        te.wait_ge(sd, 16)       # identity loaded
        for t in range(NB):
            te.wait_ge(sv, SVB * t + 1)            # IDX ready
            for k in range(b):
                if k >= 2:
                    te.wait_ge(sv, SVB * t + 1 + (k - 1))   # bank freed
                te.transpose(pT[k % 2][:], IDX[:, 128 * k:128 * (k + 1)],
                             ident[:]).then_inc(st, 1)
            te.wait_ge(sv, SVB * t + b + 2)        # enc2 ready
            for i in range(b):
                if i >= 2:
                    te.wait_ge(sv, SVB * t + b + 2 + (i - 1))
                te.transpose(pE[i % 2][:], enc2[:, i * 32:(i + 1) * 32],
                             ident[:]).then_inc(st, 1)
            te.wait_ge(sv, SVB * (t + 1))          # encT complete
            for ch in range(n_ch):
                if ch >= 2:
                    te.wait_ge(sa, SAB * t + 2 * (ch - 2) + 1)
                te.matmul(hps[ch % 2][:], w1t[:],
                          encT[:, ch * CH:(ch + 1) * CH],
                          start=True, stop=True).then_inc(st, 1)
                te.wait_ge(sa, SAB * t + 2 * ch + 1)
                te.matmul(ops[ch % 2][:], w2t[:], hsb[ch % 2][:],
                          start=True, stop=True).then_inc(st, 1)

    @block.gpsimd
    def _(g):
        # chunked gathers: 512-descriptor instructions pipeline SWDGE
        # descriptor generation with SDMA drain (4096-desc instructions
        # serialize on the ring and run ~35% slower end to end)
        wc = 16                  # offset columns per instruction
        n_sub = w // wc
        for t in range(NB):
            g.wait_ge(sv, SVB * t + 1 + b)         # O complete
            if t > 0:
                g.wait_ge(sv, SVB * (t - 1) + b + 2)  # lerps of t-1 read G
            for j in range(128):
                for c in range(n_sub):
                    g.indirect_dma_start(
                        out=G[j:j + 1,
                              c * wc * 128 * F:(c + 1) * wc * 128 * F
                              ].rearrange("p (k e) -> p k e", e=F),
                        out_offset=None,
                        in_=tabv,
                        in_offset=bass.IndirectOffsetOnAxis(
                            ap=O[:, j * w + c * wc:j * w + (c + 1) * wc],
                            axis=0),
                    ).then_inc(sg, 16)

    @block.scalar
    def _(ac):
        for t in range(NB):
            if t > 0:
                ac.wait_ge(sd, 16 * (6 + 2 * t) - 16)  # outb shipped (WAR)
            for ch in range(n_ch):
                ac.wait_ge(st, STB * t + 2 * b + 2 * ch + 1)
                ac.activation(hsb[ch % 2][:], hps[ch % 2][:],
                              AFT.Relu).then_inc(sa, 1)
                ac.wait_ge(st, STB * t + 2 * b + 2 * ch + 2)
                ac.activation(outb[:, ch * CH:(ch + 1) * CH], ops[ch % 2][:],
                              AFT.Sigmoid).then_inc(sa, 1)

    for cm in reversed(ctx):
        cm.__exit__(None, None, None)
    return nc


# ---------------- host side ----------------

class _Runner:
    def __init__(self, nc, n_cores):
        import jax
        import numpy as _np
        from jax.sharding import Mesh, PartitionSpec
        from jax.experimental.shard_map import shard_map
        import concourse.mybir as mybir
        from concourse.bass2jax import (
            install_neuronx_cc_hook, _bass_exec_p, partition_id_tensor)
        install_neuronx_cc_hook()
        self.n_cores = n_cores
        pname = nc.partition_id_tensor.name if nc.partition_id_tensor else None
        in_names, out_names, out_avals, zero_outs = [], [], [], []
        for alloc in nc.m.functions[0].allocations:
            if not isinstance(alloc, mybir.MemoryLocationSet):
                continue
            name = alloc.memorylocations[0].name
            if alloc.kind == "ExternalInput":
                if name != pname:
                    in_names.append(name)
            elif alloc.kind == "ExternalOutput":
                shape = tuple(alloc.tensor_shape)
                dtype = mybir.dt.np(alloc.dtype)
                out_names.append(name)
                out_avals.append(jax.core.ShapedArray(shape, dtype))
                zero_outs.append(_np.zeros(shape, dtype))
        self.in_names, self.out_names = in_names, out_names
        self.out_avals, self.zero_outs = out_avals, zero_outs
        n_params, n_outs = len(in_names), len(out_names)
        all_in = in_names + out_names + ([pname] if pname else [])

        def _body(*args):
            operands = list(args)
            if pname is not None:
                operands.append(partition_id_tensor())
            return tuple(_bass_exec_p.bind(
                *operands, out_avals=tuple(out_avals), in_names=tuple(all_in),
                out_names=tuple(out_names), lowering_input_output_aliases=(),
                sim_require_finite=True, sim_require_nnan=True, nc=nc))

        self.n_params, self.n_outs = n_params, n_outs
        donate = tuple(range(n_params, n_params + n_outs))
        devices = jax.devices()[:n_cores]
        mesh = Mesh(_np.asarray(devices), ("core",))
        specs = (PartitionSpec("core"),)
        self.fn = jax.jit(
            shard_map(_body, mesh=mesh, in_specs=specs * (n_params + n_outs),
                      out_specs=specs * n_outs, check_rep=False),
            donate_argnums=donate, keep_unused=True)

    def __call__(self, in_maps):
        import numpy as _np
        n = self.n_cores
        per_core = [[_np.asarray(m[nm]) for nm in self.in_names]
                    for m in in_maps]
        concat_in = [_np.concatenate([per_core[c][i] for c in range(n)], axis=0)
                     for i in range(self.n_params)]
        concat_zeros = [_np.zeros((n * z.shape[0], *z.shape[1:]), z.dtype)
                        for z in self.zero_outs]
        outs = self.fn(*concat_in, *concat_zeros)
        return [
            {nm: _np.asarray(outs[i]).reshape(n, *self.out_avals[i].shape)[c]
             for i, nm in enumerate(self.out_names)}
            for c in range(n)
        ]


_RUNNERS = {}


def _get_runner(NB, b):
    key = (NB, b)
    if key not in _RUNNERS:
        _RUNNERS[key] = _Runner(build_nc(NB, b), N_CORES)
    return _RUNNERS[key]


def _consts(b):
    Lb = L * b
    cfw = np.zeros((128, 11 * Lb), np.float32)
    ciw = np.zeros((128, 7 * Lb), np.int32)
    r1 = np.where(DENSEL, RES + 1, 0)

    def setf(slot, vals):
        cfw[:, slot * Lb:(slot + 1) * Lb] = np.repeat(
            np.asarray(vals, np.float64), b)[None, :]

    def seti(slot, vals):
        ciw[:, slot * Lb:(slot + 1) * Lb] = np.repeat(
            np.asarray(vals, np.int64), b).astype(np.int32)[None, :]

    setf(0, RES)
    setf(1, r1)
    setf(2, r1 * r1)
    for c in range(8):
        dx, dy, dz = (c >> 2) & 1, (c >> 1) & 1, c & 1
        setf(3 + c, np.where(DENSEL,
                             dx + r1 * dy + r1 * r1 * dz + np.arange(L) * T, 0))
    z = np.zeros(L, np.int64)

    def hv(val):
        a = z.copy(); a[N_DENSE:] = val; return a

    seti(0, hv(P2h)); seti(1, hv(P2l)); seti(2, hv(P3h)); seti(3, hv(P3l))
    seti(4, hv(P2p)); seti(5, hv(P3p))
    seti(6, np.arange(L) * T)
    return cfw, ciw


def _prep_core_inputs(points_core, tabflat, w1t, w2t, cfw, ciw, NB, b):
    # pts layout: [128, NB, 3, L, b]; point (t, p, i) coord d replicated /level
    p4 = points_core.reshape(NB, 128, b, 3).transpose(1, 0, 3, 2)  # p t d i
    p5 = np.repeat(p4[:, :, :, None, :], L, axis=3)                # p t d l i
    pts = np.ascontiguousarray(p5, np.float32).reshape(128, NB * 3 * L * b)
    return {"pts": pts, "tab": tabflat, "w1t": w1t, "w2t": w2t,
            "cfw": cfw, "ciw": ciw, "idm": np.eye(128, dtype=np.float32)}


def kernel(points, table, w1, w2):
    points = np.asarray(points, np.float32)
    table = np.asarray(table, np.float32)
    tabflat = np.ascontiguousarray(table.reshape(L * T * F))
    w1t = np.ascontiguousarray(np.asarray(w1, np.float32).T)
    w2t = np.ascontiguousarray(np.asarray(w2, np.float32).T)
    NB, b = N_BATCHES, B_PER_PART
    cfw, ciw = _consts(b)
    runner = _get_runner(NB, b)
    in_maps = [
        _prep_core_inputs(points[c * PTS_PER_CORE:(c + 1) * PTS_PER_CORE],
                          tabflat, w1t, w2t, cfw, ciw, NB, b)
        for c in range(N_CORES)
    ]
    res = runner(in_maps)
    outs = [res[c]["out"].reshape(-1) for c in range(N_CORES)]
    return np.concatenate(outs).reshape(1, 64, 64, 64).astype(np.float32)



# revision 11
# speedup vs baseline: 1.3689x; 1.1478x over previous
"""Instant-NGP HashGrid voxel kernel for 8 Trainium2 NeuronCores (Bass).

Data-parallel over points: each core processes N/8 = 32768 points, hash
table + MLP weights replicated. Per batch: DVE computes all 128 corner
indices per point (hash via exact split-multiply int32 ops), PE transposes
the index tile into the column-wrapped order the SWDGE indirect-DMA
consumes, 128 indirect gathers (one per dest partition) fetch 8-byte
entries, DVE does the trilinear lerp tree fused across levels, PE runs the
32->64->1 MLP with relu/sigmoid on ScalarE. Raw-Block manual semaphores
(Tile's attached multi-waits break walrus codegen here).
"""
import sys
sys.path.insert(0, "/opt/trn_rl_repo")
import numpy as np

L = 16
F = 2
T = 1 << 19
MASKC = T - 1
BASE = 16
SCALE = 1.447269237440378
N_PTS = 64 * 64 * 64
P2 = 2654435761
P3 = 805459861

RES = np.floor(BASE * SCALE ** np.arange(L) + 1e-6).astype(np.int64)
DENSEL = (RES + 1) ** 3 <= T
N_DENSE = int(DENSEL.sum())
N_HASH = L - N_DENSE

P2p, P3p = P2 & MASKC, P3 & MASKC
P2h, P2l = P2p >> 7, P2p & 127
P3h, P3l = P3p >> 7, P3p & 127

N_CORES = 8
PTS_PER_CORE = N_PTS // N_CORES
N_BATCHES = 8
B_PER_PART = 32


def build_nc(NB=N_BATCHES, b=B_PER_PART, debug=False):
    import concourse.bass as bass
    import concourse.mybir as mybir

    fp32 = mybir.dt.float32
    i32 = mybir.dt.int32
    AOT = mybir.AluOpType
    AFT = mybir.ActivationFunctionType
    Bpts = 128 * b
    W = 8 * L * b            # idx cols per partition
    w = b                    # cols per gather window (W/128)
    nb = N_DENSE * b
    nh = N_HASH * b
    Lb = L * b
    Lb2 = L * b * F
    CH = min(512, Bpts)
    n_ch = Bpts // CH
    nc = bass.Bass()

    pts_in = nc.declare_dram_parameter("pts", [128, NB * 3 * Lb], fp32, isOutput=False)
    tab = nc.declare_dram_parameter("tab", [L * T * F], fp32, isOutput=False)
    w1t_in = nc.declare_dram_parameter("w1t", [32, 64], fp32, isOutput=False)
    w2t_in = nc.declare_dram_parameter("w2t", [64, 1], fp32, isOutput=False)
    cfw_in = nc.declare_dram_parameter("cfw", [128, 11 * Lb], fp32, isOutput=False)
    ciw_in = nc.declare_dram_parameter("ciw", [128, 7 * Lb], i32, isOutput=False)
    id_in = nc.declare_dram_parameter("idm", [128, 128], fp32, isOutput=False)
    out = nc.declare_dram_parameter("out", [NB, Bpts], fp32, isOutput=True)
    if debug:
        dIDX = nc.declare_dram_parameter("dIDX", [128, W], fp32, isOutput=True)
        dO = nc.declare_dram_parameter("dO", [128, W], i32, isOutput=True)
        dG = nc.declare_dram_parameter("dG", [128, W * F], fp32, isOutput=True)
        dE2 = nc.declare_dram_parameter("dE2", [128, Lb2], fp32, isOutput=True)
        dET = nc.declare_dram_parameter("dET", [32, Bpts], fp32, isOutput=True)

    tabv = tab[:].rearrange("(t f) -> t f", f=F)

    ctx = []

    def sb(shape, dt):
        cm = nc.sbuf_tensor(shape, dt)
        t_ = cm.__enter__(); ctx.append(cm); return t_

    def ps(shape, dt):
        cm = nc.psum_tensor(shape, dt)
        t_ = cm.__enter__(); ctx.append(cm); return t_

    ident = sb([128, 128], fp32)
    w1t = sb([32, 64], fp32)
    w2t = sb([64, 1], fp32)
    cfw = sb([128, 11 * Lb], fp32)
    ciw = sb([128, 7 * Lb], i32)
    ptsb = sb([128, 3 * Lb], fp32)
    pos = [sb([128, Lb], fp32) for _ in range(3)]
    ci = [sb([128, Lb], i32) for _ in range(3)]
    c0f = [sb([128, Lb], fp32) for _ in range(3)]
    frF = [sb([128, Lb2], fp32) for _ in range(3)]
    x1h = sb([128, Lb], i32)
    yP0 = sb([128, nh], i32); yP1 = sb([128, nh], i32)
    zP0 = sb([128, nh], i32); zP1 = sb([128, nh], i32)
    tmpi = sb([128, nh], i32)
    hyz = {k: sb([128, nh], i32) for k in range(4)}
    hidx = sb([128, nh], i32)
    dbase = sb([128, nb], fp32)
    dtmp = sb([128, nb], fp32)
    IDX = sb([128, W], fp32)
    O = sb([128, W], i32)
    G = sb([128, W * F], fp32)
    tmpf = sb([128, Lb2], fp32)
    encl = sb([128, Lb2], fp32)      # (l i f)
    enc2 = sb([128, Lb2], fp32)      # (i l f)
    encT = sb([32, Bpts], fp32)
    hsb = [sb([64, CH], fp32) for _ in range(2)]
    outb = sb([1, Bpts], fp32)
    pT = [ps([128, 128], fp32) for _ in range(2)]
    pE = [ps([32, 128], fp32) for _ in range(2)]
    hps = [ps([64, CH], fp32) for _ in range(2)]
    ops = [ps([1, CH], fp32) for _ in range(2)]

    sd_cm = nc.semaphore(); sd = sd_cm.__enter__(); ctx.append(sd_cm)
    sg_cm = nc.semaphore(); sg = sg_cm.__enter__(); ctx.append(sg_cm)
    sj_cm = nc.semaphore(); sj = sj_cm.__enter__(); ctx.append(sj_cm)
    sv_cm = nc.semaphore(); sv = sv_cm.__enter__(); ctx.append(sv_cm)
    st_cm = nc.semaphore(); st = st_cm.__enter__(); ctx.append(st_cm)
    sa_cm = nc.semaphore(); sa = sa_cm.__enter__(); ctx.append(sa_cm)

    NCONST = 7          # const DMAs
    GPB = 16 * 128                # gather sem increments per batch (1/row)
    STB = b + b + 2 * n_ch        # tensor instrs per batch
    SVB = 1 + b + 1 + b           # vector sem incs per batch
    SAB = 2 * n_ch
    Or = O[:].rearrange("p (j k) -> p k j", k=w)
    eTr = encT[:].rearrange("q (P m) -> q m P", m=b)

    def cslice(tile_, slot, hash_only=False, dense_only=False):
        s = slot * Lb
        if hash_only:
            return tile_[:, s + nb: s + Lb]
        if dense_only:
            return tile_[:, s: s + nb]
        return tile_[:, s: s + Lb]

    blk_cm = nc.Block(); block = blk_cm.__enter__(); ctx.append(blk_cm)

    @block.sync
    def _(sy):
        sy.dma_start(ident[:], id_in[:]).then_inc(sd, 16)
        sy.dma_start(w1t[:], w1t_in[:]).then_inc(sd, 16)
        sy.dma_start(w2t[:], w2t_in[:]).then_inc(sd, 16)
        sy.dma_start(cfw[:], cfw_in[:]).then_inc(sd, 16)
        sy.dma_start(ciw[:], ciw_in[:]).then_inc(sd, 16)
        sy.dma_start(ptsb[:], pts_in[:, 0:3 * Lb]).then_inc(sd, 16)
        for t in range(NB):
            # wait batch t fully written by scalar, then ship out + next pts
            sy.wait_ge(sa, SAB * (t + 1))
            sy.dma_start(out[t:t + 1, :], outb[:]).then_inc(sd, 16)
            if t + 1 < NB:
                sy.dma_start(
                    ptsb[:], pts_in[:, (t + 1) * 3 * Lb:(t + 2) * 3 * Lb]
                ).then_inc(sd, 16)
        if debug:
            sy.dma_start(dIDX[:], IDX[:]).then_inc(sd, 16)
            sy.dma_start(dO[:], O[:]).then_inc(sd, 16)
            sy.dma_start(dG[:], G[:]).then_inc(sd, 16)
            sy.dma_start(dE2[:], enc2[:]).then_inc(sd, 16)
            sy.dma_start(dET[:], encT[:]).then_inc(sd, 16)
            sy.wait_ge(sd, 16 * (6 + 2 * NB - 2) + 80)

    @block.vector
    def _(v):
        for t in range(NB):
            # pts batch ready (NCONST-1 consts + t-th ptsb; out DMAs interleave)
            v.wait_ge(sd, 16 * (6 + 2 * t))
            if t > 0:
                v.wait_ge(st, STB * t)      # tensor done reading IDX/enc/encT
            # ---- floors / fracs ----
            for d in range(3):
                pd = ptsb[:, d * Lb:(d + 1) * Lb]
                v.tensor_tensor(out=pos[d][:], in0=pd, in1=cslice(cfw, 0),
                                op=AOT.mult)
                v.tensor_scalar(out=pos[d][:], in0=pos[d][:], scalar1=-0.5,
                                scalar2=None, op0=AOT.add)
                v.tensor_copy(out=ci[d][:], in_=pos[d][:])
                v.tensor_copy(out=c0f[d][:], in_=ci[d][:])
                # frac = (pos-0.5 - c0f) + 0.5 stored duplicated over feats
                v.tensor_tensor(out=pos[d][:], in0=pos[d][:], in1=c0f[d][:],
                                op=AOT.subtract)
                v.tensor_scalar(out=pos[d][:], in0=pos[d][:], scalar1=0.5,
                                scalar2=None, op0=AOT.add)
                fv = frF[d][:].rearrange("p (x e) -> p x e", e=F)
                v.tensor_copy(out=fv[:, :, 0], in_=pos[d][:])
                v.tensor_copy(out=fv[:, :, 1], in_=pos[d][:])
            # ---- hash products ----
            for (dst, srcci, hi, lo) in ((yP0, ci[1], 0, 1), (zP0, ci[2], 2, 3)):
                v.tensor_tensor(out=dst[:], in0=srcci[:, nb:Lb],
                                in1=cslice(ciw, hi, hash_only=True), op=AOT.mult)
                v.tensor_scalar(out=dst[:], in0=dst[:], scalar1=7, scalar2=None,
                                op0=AOT.logical_shift_left)
                v.tensor_tensor(out=tmpi[:], in0=srcci[:, nb:Lb],
                                in1=cslice(ciw, lo, hash_only=True), op=AOT.mult)
                v.tensor_tensor(out=dst[:], in0=dst[:], in1=tmpi[:], op=AOT.add)
            v.tensor_tensor(out=yP1[:], in0=yP0[:],
                            in1=cslice(ciw, 4, hash_only=True), op=AOT.add)
            v.tensor_tensor(out=zP1[:], in0=zP0[:],
                            in1=cslice(ciw, 5, hash_only=True), op=AOT.add)
            for dy, yy in ((0, yP0), (1, yP1)):
                for dz, zz in ((0, zP0), (1, zP1)):
                    v.tensor_tensor(out=hyz[dy * 2 + dz][:], in0=yy[:],
                                    in1=zz[:], op=AOT.bitwise_xor)
            v.tensor_scalar(out=x1h[:], in0=ci[0][:], scalar1=1, scalar2=None,
                            op0=AOT.add)
            for c in range(8):
                dx, dy, dz = (c >> 2) & 1, (c >> 1) & 1, c & 1
                xx = x1h if dx else ci[0]
                v.tensor_tensor(out=hidx[:], in0=xx[:, nb:Lb],
                                in1=hyz[dy * 2 + dz][:], op=AOT.bitwise_xor)
                v.tensor_scalar(out=hidx[:], in0=hidx[:], scalar1=MASKC,
                                scalar2=None, op0=AOT.bitwise_and)
                v.tensor_tensor(out=hidx[:], in0=hidx[:],
                                in1=cslice(ciw, 6, hash_only=True), op=AOT.add)
                v.tensor_copy(out=IDX[:, c * Lb + nb:(c + 1) * Lb], in_=hidx[:])
            # ---- dense indices (float, exact) ----
            v.tensor_tensor(out=dbase[:], in0=c0f[1][:, 0:nb],
                            in1=cslice(cfw, 1, dense_only=True), op=AOT.mult)
            v.tensor_tensor(out=dbase[:], in0=dbase[:], in1=c0f[0][:, 0:nb],
                            op=AOT.add)
            v.tensor_tensor(out=dtmp[:], in0=c0f[2][:, 0:nb],
                            in1=cslice(cfw, 2, dense_only=True), op=AOT.mult)
            v.tensor_tensor(out=dbase[:], in0=dbase[:], in1=dtmp[:], op=AOT.add)
            for c in range(8):
                v.tensor_tensor(out=IDX[:, c * Lb:c * Lb + nb], in0=dbase[:],
                                in1=cslice(cfw, 3 + c, dense_only=True),
                                op=AOT.add)

            v.tensor_copy(out=tmpi[:, 0:1], in_=tmpi[:, 0:1]).then_inc(sv, 1)
            # ---- copy PE-transposed IDX blocks into O ----
            if t > 0:
                v.wait_ge(sg, GPB * t)       # gathers of prev batch done (WAR O)
            for k in range(b):
                v.wait_ge(st, STB * t + k + 1)
                v.tensor_copy(out=Or[:, k, :], in_=pT[k % 2][:]).then_inc(sv, 1)
            # ---- wait gathers, lerp ----
            v.wait_ge(sg, GPB * (t + 1))

            def gc(c):
                return G[:, c * Lb2:(c + 1) * Lb2]

            for c in (0, 2, 4, 6):
                v.tensor_tensor(out=tmpf[:], in0=gc(c + 1), in1=gc(c),
                                op=AOT.subtract)
                v.tensor_tensor(out=tmpf[:], in0=tmpf[:], in1=frF[2][:],
                                op=AOT.mult)
                v.tensor_tensor(out=gc(c), in0=gc(c), in1=tmpf[:], op=AOT.add)
            for c in (0, 4):
                v.tensor_tensor(out=tmpf[:], in0=gc(c + 2), in1=gc(c),
                                op=AOT.subtract)
                v.tensor_tensor(out=tmpf[:], in0=tmpf[:], in1=frF[1][:],
                                op=AOT.mult)
                v.tensor_tensor(out=gc(c), in0=gc(c), in1=tmpf[:], op=AOT.add)
            v.tensor_tensor(out=tmpf[:], in0=gc(4), in1=gc(0), op=AOT.subtract)
            v.tensor_tensor(out=tmpf[:], in0=tmpf[:], in1=frF[0][:],
                            op=AOT.mult)
            v.tensor_tensor(out=encl[:], in0=gc(0), in1=tmpf[:], op=AOT.add)
            # reorder (l i f) -> (i l f): one strided copy per level
            for l in range(L):
                src = encl[:, l * b * F:(l + 1) * b * F].rearrange(
                    "p (i e) -> p i e", e=F)
                dst = enc2[:].rearrange("p (i l e) -> p i l e", l=L, e=F)[:, :, l, :]
                v.tensor_copy(out=dst, in_=src)
            v.tensor_copy(out=tmpi[:, 0:1], in_=tmpi[:, 0:1]).then_inc(sv, 1)
            # ---- copy PE-transposed enc blocks into encT ----
            for i in range(b):
                v.wait_ge(st, STB * t + b + i + 1)
                v.tensor_copy(out=eTr[:, i, :], in_=pE[i % 2][:]).then_inc(sv, 1)

    @block.tensor
    def _(te):
        te.wait_ge(sd, 16)       # identity loaded
        for t in range(NB):
            te.wait_ge(sv, SVB * t + 1)            # IDX ready
            for k in range(b):
                if k >= 2:
                    te.wait_ge(sv, SVB * t + 1 + (k - 1))   # bank freed
                te.transpose(pT[k % 2][:], IDX[:, 128 * k:128 * (k + 1)],
                             ident[:]).then_inc(st, 1)
            te.wait_ge(sv, SVB * t + b + 2)        # enc2 ready
            for i in range(b):
                if i >= 2:
                    te.wait_ge(sv, SVB * t + b + 2 + (i - 1))
                te.transpose(pE[i % 2][:], enc2[:, i * 32:(i + 1) * 32],
                             ident[:]).then_inc(st, 1)
            te.wait_ge(sv, SVB * (t + 1))          # encT complete
            for ch in range(n_ch):
                if ch >= 2:
                    te.wait_ge(sa, SAB * t + 2 * (ch - 2) + 1)
                te.matmul(hps[ch % 2][:], w1t[:],
                          encT[:, ch * CH:(ch + 1) * CH],
                          start=True, stop=True).then_inc(st, 1)
                te.wait_ge(sa, SAB * t + 2 * ch + 1)
                te.matmul(ops[ch % 2][:], w2t[:], hsb[ch % 2][:],
                          start=True, stop=True).then_inc(st, 1)

    @block.gpsimd
    def _(g):
        # chunked gathers: 512-descriptor instructions pipeline SWDGE
        # descriptor generation with SDMA drain (4096-desc instructions
        # serialize on the ring and run ~35% slower end to end)
        wc = 4                   # offset columns per instruction
        n_sub = w // wc
        for t in range(NB):
            g.wait_ge(sv, SVB * t + 1 + b)         # O complete
            if t > 0:
                g.wait_ge(sv, SVB * (t - 1) + b + 2)  # lerps of t-1 read G
            for j in range(128):
                for c in range(n_sub):
                    inst = g.indirect_dma_start(
                        out=G[j:j + 1,
                              c * wc * 128 * F:(c + 1) * wc * 128 * F
                              ].rearrange("p (k e) -> p k e", e=F),
                        out_offset=None,
                        in_=tabv,
                        in_offset=bass.IndirectOffsetOnAxis(
                            ap=O[:, j * w + c * wc:j * w + (c + 1) * wc],
                            axis=0),
                    )
                    # tracked sem only on each row's last chunk: per-engine
                    # ring FIFO makes it cover the row's earlier chunks, and
                    # the tracked total stays within the counter range.
                    # intermediate chunks get an unwatched sem (walrus
                    # requires sync info on every DGE instruction)
                    if c == n_sub - 1:
                        inst.then_inc(sg, 16)
                    else:
                        inst.then_inc(sj, 16)

    @block.scalar
    def _(ac):
        for t in range(NB):
            if t > 0:
                ac.wait_ge(sd, 16 * (6 + 2 * t) - 16)  # outb shipped (WAR)
            for ch in range(n_ch):
                ac.wait_ge(st, STB * t + 2 * b + 2 * ch + 1)
                ac.activation(hsb[ch % 2][:], hps[ch % 2][:],
                              AFT.Relu).then_inc(sa, 1)
                ac.wait_ge(st, STB * t + 2 * b + 2 * ch + 2)
                ac.activation(outb[:, ch * CH:(ch + 1) * CH], ops[ch % 2][:],
                              AFT.Sigmoid).then_inc(sa, 1)

    for cm in reversed(ctx):
        cm.__exit__(None, None, None)
    return nc


# ---------------- host side ----------------

class _Runner:
    def __init__(self, nc, n_cores):
        import jax
        import numpy as _np
        from jax.sharding import Mesh, PartitionSpec
        from jax.experimental.shard_map import shard_map
        import concourse.mybir as mybir
        from concourse.bass2jax import (
            install_neuronx_cc_hook, _bass_exec_p, partition_id_tensor)
        install_neuronx_cc_hook()
        self.n_cores = n_cores
        pname = nc.partition_id_tensor.name if nc.partition_id_tensor else None
        in_names, out_names, out_avals, zero_outs = [], [], [], []
        for alloc in nc.m.functions[0].allocations:
            if not isinstance(alloc, mybir.MemoryLocationSet):
                continue
            name = alloc.memorylocations[0].name
            if alloc.kind == "ExternalInput":
                if name != pname:
                    in_names.append(name)
            elif alloc.kind == "ExternalOutput":
                shape = tuple(alloc.tensor_shape)
                dtype = mybir.dt.np(alloc.dtype)
                out_names.append(name)
                out_avals.append(jax.core.ShapedArray(shape, dtype))
                zero_outs.append(_np.zeros(shape, dtype))
        self.in_names, self.out_names = in_names, out_names
        self.out_avals, self.zero_outs = out_avals, zero_outs
        n_params, n_outs = len(in_names), len(out_names)
        all_in = in_names + out_names + ([pname] if pname else [])

        def _body(*args):
            operands = list(args)
            if pname is not None:
                operands.append(partition_id_tensor())
            return tuple(_bass_exec_p.bind(
                *operands, out_avals=tuple(out_avals), in_names=tuple(all_in),
                out_names=tuple(out_names), lowering_input_output_aliases=(),
                sim_require_finite=True, sim_require_nnan=True, nc=nc))

        self.n_params, self.n_outs = n_params, n_outs
        donate = tuple(range(n_params, n_params + n_outs))
        devices = jax.devices()[:n_cores]
        mesh = Mesh(_np.asarray(devices), ("core",))
        specs = (PartitionSpec("core"),)
        self.fn = jax.jit(
            shard_map(_body, mesh=mesh, in_specs=specs * (n_params + n_outs),
                      out_specs=specs * n_outs, check_rep=False),
            donate_argnums=donate, keep_unused=True)

    def __call__(self, in_maps):
        import numpy as _np
        n = self.n_cores
        per_core = [[_np.asarray(m[nm]) for nm in self.in_names]
                    for m in in_maps]
        concat_in = [_np.concatenate([per_core[c][i] for c in range(n)], axis=0)
                     for i in range(self.n_params)]
        concat_zeros = [_np.zeros((n * z.shape[0], *z.shape[1:]), z.dtype)
                        for z in self.zero_outs]
        outs = self.fn(*concat_in, *concat_zeros)
        return [
            {nm: _np.asarray(outs[i]).reshape(n, *self.out_avals[i].shape)[c]
             for i, nm in enumerate(self.out_names)}
            for c in range(n)
        ]


_RUNNERS = {}


def _get_runner(NB, b):
    key = (NB, b)
    if key not in _RUNNERS:
        _RUNNERS[key] = _Runner(build_nc(NB, b), N_CORES)
    return _RUNNERS[key]


def _consts(b):
    Lb = L * b
    cfw = np.zeros((128, 11 * Lb), np.float32)
    ciw = np.zeros((128, 7 * Lb), np.int32)
    r1 = np.where(DENSEL, RES + 1, 0)

    def setf(slot, vals):
        cfw[:, slot * Lb:(slot + 1) * Lb] = np.repeat(
            np.asarray(vals, np.float64), b)[None, :]

    def seti(slot, vals):
        ciw[:, slot * Lb:(slot + 1) * Lb] = np.repeat(
            np.asarray(vals, np.int64), b).astype(np.int32)[None, :]

    setf(0, RES)
    setf(1, r1)
    setf(2, r1 * r1)
    for c in range(8):
        dx, dy, dz = (c >> 2) & 1, (c >> 1) & 1, c & 1
        setf(3 + c, np.where(DENSEL,
                             dx + r1 * dy + r1 * r1 * dz + np.arange(L) * T, 0))
    z = np.zeros(L, np.int64)

    def hv(val):
        a = z.copy(); a[N_DENSE:] = val; return a

    seti(0, hv(P2h)); seti(1, hv(P2l)); seti(2, hv(P3h)); seti(3, hv(P3l))
    seti(4, hv(P2p)); seti(5, hv(P3p))
    seti(6, np.arange(L) * T)
    return cfw, ciw


def _prep_core_inputs(points_core, tabflat, w1t, w2t, cfw, ciw, NB, b):
    # pts layout: [128, NB, 3, L, b]; point (t, p, i) coord d replicated /level
    p4 = points_core.reshape(NB, 128, b, 3).transpose(1, 0, 3, 2)  # p t d i
    p5 = np.repeat(p4[:, :, :, None, :], L, axis=3)                # p t d l i
    pts = np.ascontiguousarray(p5, np.float32).reshape(128, NB * 3 * L * b)
    return {"pts": pts, "tab": tabflat, "w1t": w1t, "w2t": w2t,
            "cfw": cfw, "ciw": ciw, "idm": np.eye(128, dtype=np.float32)}


def kernel(points, table, w1, w2):
    points = np.asarray(points, np.float32)
    table = np.asarray(table, np.float32)
    tabflat = np.ascontiguousarray(table.reshape(L * T * F))
    w1t = np.ascontiguousarray(np.asarray(w1, np.float32).T)
    w2t = np.ascontiguousarray(np.asarray(w2, np.float32).T)
    NB, b = N_BATCHES, B_PER_PART
    cfw, ciw = _consts(b)
    runner = _get_runner(NB, b)
    in_maps = [
        _prep_core_inputs(points[c * PTS_PER_CORE:(c + 1) * PTS_PER_CORE],
                          tabflat, w1t, w2t, cfw, ciw, NB, b)
        for c in range(N_CORES)
    ]
    res = runner(in_maps)
    outs = [res[c]["out"].reshape(-1) for c in range(N_CORES)]
    return np.concatenate(outs).reshape(1, 64, 64, 64).astype(np.float32)



# revision 14
# speedup vs baseline: 1.5095x; 1.1027x over previous
"""Instant-NGP HashGrid voxel kernel for 8 Trainium2 NeuronCores (Bass).

Data-parallel over points: each core processes N/8 = 32768 points, hash
table + MLP weights replicated. Per batch: DVE computes all 128 corner
indices per point (hash via exact split-multiply int32 ops), PE transposes
the index tile into the column-wrapped order the SWDGE indirect-DMA
consumes, 128 indirect gathers (one per dest partition) fetch 8-byte
entries, DVE does the trilinear lerp tree fused across levels, PE runs the
32->64->1 MLP with relu/sigmoid on ScalarE. Raw-Block manual semaphores
(Tile's attached multi-waits break walrus codegen here).
"""
import sys
sys.path.insert(0, "/opt/trn_rl_repo")
import numpy as np

L = 16
F = 2
T = 1 << 19
MASKC = T - 1
BASE = 16
SCALE = 1.447269237440378
N_PTS = 64 * 64 * 64
P2 = 2654435761
P3 = 805459861

RES = np.floor(BASE * SCALE ** np.arange(L) + 1e-6).astype(np.int64)
DENSEL = (RES + 1) ** 3 <= T
N_DENSE = int(DENSEL.sum())
N_HASH = L - N_DENSE

P2p, P3p = P2 & MASKC, P3 & MASKC
P2h, P2l = P2p >> 7, P2p & 127
P3h, P3l = P3p >> 7, P3p & 127

N_CORES = 8
PTS_PER_CORE = N_PTS // N_CORES
N_BATCHES = 8
B_PER_PART = 32


def build_nc(NB=N_BATCHES, b=B_PER_PART, debug=False):
    import concourse.bass as bass
    import concourse.mybir as mybir

    fp32 = mybir.dt.float32
    i32 = mybir.dt.int32
    AOT = mybir.AluOpType
    AFT = mybir.ActivationFunctionType
    Bpts = 128 * b
    W = 8 * L * b            # idx cols per partition
    w = b                    # cols per gather window (W/128)
    nb = N_DENSE * b
    nh = N_HASH * b
    Lb = L * b
    Lb2 = L * b * F
    CH = min(512, Bpts)
    n_ch = Bpts // CH
    nc = bass.Bass(dynamic_dma_scratch_size=20480)

    pts_in = nc.declare_dram_parameter("pts", [128, NB * 3 * Lb], fp32, isOutput=False)
    tab = nc.declare_dram_parameter("tab", [L * T * F], fp32, isOutput=False)
    w1t_in = nc.declare_dram_parameter("w1t", [32, 64], fp32, isOutput=False)
    w2t_in = nc.declare_dram_parameter("w2t", [64, 1], fp32, isOutput=False)
    cfw_in = nc.declare_dram_parameter("cfw", [128, 11 * Lb], fp32, isOutput=False)
    ciw_in = nc.declare_dram_parameter("ciw", [128, 7 * Lb], i32, isOutput=False)
    id_in = nc.declare_dram_parameter("idm", [128, 128], fp32, isOutput=False)
    out = nc.declare_dram_parameter("out", [NB, Bpts], fp32, isOutput=True)
    if debug:
        dIDX = nc.declare_dram_parameter("dIDX", [128, W], fp32, isOutput=True)
        dO = nc.declare_dram_parameter("dO", [128, W], i32, isOutput=True)
        dG = nc.declare_dram_parameter("dG", [128, W * F], fp32, isOutput=True)
        dE2 = nc.declare_dram_parameter("dE2", [128, Lb2], fp32, isOutput=True)
        dET = nc.declare_dram_parameter("dET", [32, Bpts], fp32, isOutput=True)

    tabv = tab[:].rearrange("(t f) -> t f", f=F)

    ctx = []

    def sb(shape, dt):
        cm = nc.sbuf_tensor(shape, dt)
        t_ = cm.__enter__(); ctx.append(cm); return t_

    def ps(shape, dt):
        cm = nc.psum_tensor(shape, dt)
        t_ = cm.__enter__(); ctx.append(cm); return t_

    ident = sb([128, 128], fp32)
    w1t = sb([32, 64], fp32)
    w2t = sb([64, 1], fp32)
    cfw = sb([128, 11 * Lb], fp32)
    ciw = sb([128, 7 * Lb], i32)
    ptsb = sb([128, 3 * Lb], fp32)
    pos = [sb([128, Lb], fp32) for _ in range(3)]
    ci = [sb([128, Lb], i32) for _ in range(3)]
    c0f = [sb([128, Lb], fp32) for _ in range(3)]
    frF = [sb([128, Lb2], fp32) for _ in range(3)]
    x1h = sb([128, Lb], i32)
    yP0 = sb([128, nh], i32); yP1 = sb([128, nh], i32)
    zP0 = sb([128, nh], i32); zP1 = sb([128, nh], i32)
    tmpi = sb([128, nh], i32)
    hyz = {k: sb([128, nh], i32) for k in range(4)}
    hidx = sb([128, nh], i32)
    dbase = sb([128, nb], fp32)
    dtmp = sb([128, nb], fp32)
    IDX = sb([128, W], fp32)
    O = sb([128, W], i32)
    G = sb([128, W * F], fp32)
    tmpf = sb([128, Lb2], fp32)
    encl = sb([128, Lb2], fp32)      # (l i f)
    enc2 = sb([128, Lb2], fp32)      # (i l f)
    encT = sb([32, Bpts], fp32)
    hsb = [sb([64, CH], fp32) for _ in range(2)]
    outb = sb([1, Bpts], fp32)
    pT = [ps([128, 128], fp32) for _ in range(2)]
    pE = [ps([32, 128], fp32) for _ in range(2)]
    hps = [ps([64, CH], fp32) for _ in range(2)]
    ops = [ps([1, CH], fp32) for _ in range(2)]

    sd_cm = nc.semaphore(); sd = sd_cm.__enter__(); ctx.append(sd_cm)
    sg_cm = nc.semaphore(); sg = sg_cm.__enter__(); ctx.append(sg_cm)
    sj_cm = nc.semaphore(); sj = sj_cm.__enter__(); ctx.append(sj_cm)
    sv_cm = nc.semaphore(); sv = sv_cm.__enter__(); ctx.append(sv_cm)
    st_cm = nc.semaphore(); st = st_cm.__enter__(); ctx.append(st_cm)
    sa_cm = nc.semaphore(); sa = sa_cm.__enter__(); ctx.append(sa_cm)

    NCONST = 7          # const DMAs
    GPB = 16 * 128                # gather sem increments per batch (1/row)
    STB = b + b + 2 * n_ch        # tensor instrs per batch
    SVB = 1 + b + 1 + b           # vector sem incs per batch
    SAB = 2 * n_ch
    Or = O[:].rearrange("p (j k) -> p k j", k=w)
    eTr = encT[:].rearrange("q (P m) -> q m P", m=b)

    def cslice(tile_, slot, hash_only=False, dense_only=False):
        s = slot * Lb
        if hash_only:
            return tile_[:, s + nb: s + Lb]
        if dense_only:
            return tile_[:, s: s + nb]
        return tile_[:, s: s + Lb]

    blk_cm = nc.Block(); block = blk_cm.__enter__(); ctx.append(blk_cm)

    @block.sync
    def _(sy):
        sy.dma_start(ident[:], id_in[:]).then_inc(sd, 16)
        sy.dma_start(w1t[:], w1t_in[:]).then_inc(sd, 16)
        sy.dma_start(w2t[:], w2t_in[:]).then_inc(sd, 16)
        sy.dma_start(cfw[:], cfw_in[:]).then_inc(sd, 16)
        sy.dma_start(ciw[:], ciw_in[:]).then_inc(sd, 16)
        sy.dma_start(ptsb[:], pts_in[:, 0:3 * Lb]).then_inc(sd, 16)
        for t in range(NB):
            # wait batch t fully written by scalar, then ship out + next pts
            sy.wait_ge(sa, SAB * (t + 1))
            sy.dma_start(out[t:t + 1, :], outb[:]).then_inc(sd, 16)
            if t + 1 < NB:
                sy.dma_start(
                    ptsb[:], pts_in[:, (t + 1) * 3 * Lb:(t + 2) * 3 * Lb]
                ).then_inc(sd, 16)
        if debug:
            sy.dma_start(dIDX[:], IDX[:]).then_inc(sd, 16)
            sy.dma_start(dO[:], O[:]).then_inc(sd, 16)
            sy.dma_start(dG[:], G[:]).then_inc(sd, 16)
            sy.dma_start(dE2[:], enc2[:]).then_inc(sd, 16)
            sy.dma_start(dET[:], encT[:]).then_inc(sd, 16)
            sy.wait_ge(sd, 16 * (6 + 2 * NB - 2) + 80)

    @block.vector
    def _(v):
        for t in range(NB):
            # pts batch ready (NCONST-1 consts + t-th ptsb; out DMAs interleave)
            v.wait_ge(sd, 16 * (6 + 2 * t))
            if t > 0:
                v.wait_ge(st, STB * t)      # tensor done reading IDX/enc/encT
            # ---- floors / fracs ----
            for d in range(3):
                pd = ptsb[:, d * Lb:(d + 1) * Lb]
                v.tensor_tensor(out=pos[d][:], in0=pd, in1=cslice(cfw, 0),
                                op=AOT.mult)
                v.tensor_scalar(out=pos[d][:], in0=pos[d][:], scalar1=-0.5,
                                scalar2=None, op0=AOT.add)
                v.tensor_copy(out=ci[d][:], in_=pos[d][:])
                v.tensor_copy(out=c0f[d][:], in_=ci[d][:])
                # frac = (pos-0.5 - c0f) + 0.5 stored duplicated over feats
                v.tensor_tensor(out=pos[d][:], in0=pos[d][:], in1=c0f[d][:],
                                op=AOT.subtract)
                v.tensor_scalar(out=pos[d][:], in0=pos[d][:], scalar1=0.5,
                                scalar2=None, op0=AOT.add)
                fv = frF[d][:].rearrange("p (x e) -> p x e", e=F)
                v.tensor_copy(out=fv[:, :, 0], in_=pos[d][:])
                v.tensor_copy(out=fv[:, :, 1], in_=pos[d][:])
            # ---- hash products ----
            for (dst, srcci, hi, lo) in ((yP0, ci[1], 0, 1), (zP0, ci[2], 2, 3)):
                v.tensor_tensor(out=dst[:], in0=srcci[:, nb:Lb],
                                in1=cslice(ciw, hi, hash_only=True), op=AOT.mult)
                v.tensor_scalar(out=dst[:], in0=dst[:], scalar1=7, scalar2=None,
                                op0=AOT.logical_shift_left)
                v.tensor_tensor(out=tmpi[:], in0=srcci[:, nb:Lb],
                                in1=cslice(ciw, lo, hash_only=True), op=AOT.mult)
                v.tensor_tensor(out=dst[:], in0=dst[:], in1=tmpi[:], op=AOT.add)
            v.tensor_tensor(out=yP1[:], in0=yP0[:],
                            in1=cslice(ciw, 4, hash_only=True), op=AOT.add)
            v.tensor_tensor(out=zP1[:], in0=zP0[:],
                            in1=cslice(ciw, 5, hash_only=True), op=AOT.add)
            for dy, yy in ((0, yP0), (1, yP1)):
                for dz, zz in ((0, zP0), (1, zP1)):
                    v.tensor_tensor(out=hyz[dy * 2 + dz][:], in0=yy[:],
                                    in1=zz[:], op=AOT.bitwise_xor)
            v.tensor_scalar(out=x1h[:], in0=ci[0][:], scalar1=1, scalar2=None,
                            op0=AOT.add)
            for c in range(8):
                dx, dy, dz = (c >> 2) & 1, (c >> 1) & 1, c & 1
                xx = x1h if dx else ci[0]
                v.tensor_tensor(out=hidx[:], in0=xx[:, nb:Lb],
                                in1=hyz[dy * 2 + dz][:], op=AOT.bitwise_xor)
                v.tensor_scalar(out=hidx[:], in0=hidx[:], scalar1=MASKC,
                                scalar2=None, op0=AOT.bitwise_and)
                v.tensor_tensor(out=hidx[:], in0=hidx[:],
                                in1=cslice(ciw, 6, hash_only=True), op=AOT.add)
                v.tensor_copy(out=IDX[:, c * Lb + nb:(c + 1) * Lb], in_=hidx[:])
            # ---- dense indices (float, exact) ----
            v.tensor_tensor(out=dbase[:], in0=c0f[1][:, 0:nb],
                            in1=cslice(cfw, 1, dense_only=True), op=AOT.mult)
            v.tensor_tensor(out=dbase[:], in0=dbase[:], in1=c0f[0][:, 0:nb],
                            op=AOT.add)
            v.tensor_tensor(out=dtmp[:], in0=c0f[2][:, 0:nb],
                            in1=cslice(cfw, 2, dense_only=True), op=AOT.mult)
            v.tensor_tensor(out=dbase[:], in0=dbase[:], in1=dtmp[:], op=AOT.add)
            for c in range(8):
                v.tensor_tensor(out=IDX[:, c * Lb:c * Lb + nb], in0=dbase[:],
                                in1=cslice(cfw, 3 + c, dense_only=True),
                                op=AOT.add)

            v.tensor_copy(out=tmpi[:, 0:1], in_=tmpi[:, 0:1]).then_inc(sv, 1)
            # ---- copy PE-transposed IDX blocks into O ----
            if t > 0:
                v.wait_ge(sg, GPB * t)       # gathers of prev batch done (WAR O)
            for k in range(b):
                v.wait_ge(st, STB * t + k + 1)
                v.tensor_copy(out=Or[:, k, :], in_=pT[k % 2][:]).then_inc(sv, 1)
            # ---- wait gathers, lerp ----
            v.wait_ge(sg, GPB * (t + 1))

            def gc(c):
                return G[:, c * Lb2:(c + 1) * Lb2]

            for c in (0, 2, 4, 6):
                v.tensor_tensor(out=tmpf[:], in0=gc(c + 1), in1=gc(c),
                                op=AOT.subtract)
                v.tensor_tensor(out=tmpf[:], in0=tmpf[:], in1=frF[2][:],
                                op=AOT.mult)
                v.tensor_tensor(out=gc(c), in0=gc(c), in1=tmpf[:], op=AOT.add)
            for c in (0, 4):
                v.tensor_tensor(out=tmpf[:], in0=gc(c + 2), in1=gc(c),
                                op=AOT.subtract)
                v.tensor_tensor(out=tmpf[:], in0=tmpf[:], in1=frF[1][:],
                                op=AOT.mult)
                v.tensor_tensor(out=gc(c), in0=gc(c), in1=tmpf[:], op=AOT.add)
            v.tensor_tensor(out=tmpf[:], in0=gc(4), in1=gc(0), op=AOT.subtract)
            v.tensor_tensor(out=tmpf[:], in0=tmpf[:], in1=frF[0][:],
                            op=AOT.mult)
            v.tensor_tensor(out=encl[:], in0=gc(0), in1=tmpf[:], op=AOT.add)
            # reorder (l i f) -> (i l f): one strided copy per level
            for l in range(L):
                src = encl[:, l * b * F:(l + 1) * b * F].rearrange(
                    "p (i e) -> p i e", e=F)
                dst = enc2[:].rearrange("p (i l e) -> p i l e", l=L, e=F)[:, :, l, :]
                v.tensor_copy(out=dst, in_=src)
            v.tensor_copy(out=tmpi[:, 0:1], in_=tmpi[:, 0:1]).then_inc(sv, 1)
            # ---- copy PE-transposed enc blocks into encT ----
            for i in range(b):
                v.wait_ge(st, STB * t + b + i + 1)
                v.tensor_copy(out=eTr[:, i, :], in_=pE[i % 2][:]).then_inc(sv, 1)

    @block.tensor
    def _(te):
        te.wait_ge(sd, 16)       # identity loaded
        for t in range(NB):
            te.wait_ge(sv, SVB * t + 1)            # IDX ready
            for k in range(b):
                if k >= 2:
                    te.wait_ge(sv, SVB * t + 1 + (k - 1))   # bank freed
                te.transpose(pT[k % 2][:], IDX[:, 128 * k:128 * (k + 1)],
                             ident[:]).then_inc(st, 1)
            te.wait_ge(sv, SVB * t + b + 2)        # enc2 ready
            for i in range(b):
                if i >= 2:
                    te.wait_ge(sv, SVB * t + b + 2 + (i - 1))
                te.transpose(pE[i % 2][:], enc2[:, i * 32:(i + 1) * 32],
                             ident[:]).then_inc(st, 1)
            te.wait_ge(sv, SVB * (t + 1))          # encT complete
            for ch in range(n_ch):
                if ch >= 2:
                    te.wait_ge(sa, SAB * t + 2 * (ch - 2) + 1)
                te.matmul(hps[ch % 2][:], w1t[:],
                          encT[:, ch * CH:(ch + 1) * CH],
                          start=True, stop=True).then_inc(st, 1)
                te.wait_ge(sa, SAB * t + 2 * ch + 1)
                te.matmul(ops[ch % 2][:], w2t[:], hsb[ch % 2][:],
                          start=True, stop=True).then_inc(st, 1)

    @block.gpsimd
    def _(g):
        # chunked gathers: 512-descriptor instructions pipeline SWDGE
        # descriptor generation with SDMA drain (4096-desc instructions
        # serialize on the ring and run ~35% slower end to end)
        wc = 8                   # offset columns per instruction
        n_sub = w // wc
        for t in range(NB):
            g.wait_ge(sv, SVB * t + 1 + b)         # O complete
            if t > 0:
                g.wait_ge(sv, SVB * (t - 1) + b + 2)  # lerps of t-1 read G
            for j in range(128):
                for c in range(n_sub):
                    inst = g.indirect_dma_start(
                        out=G[j:j + 1,
                              c * wc * 128 * F:(c + 1) * wc * 128 * F
                              ].rearrange("p (k e) -> p k e", e=F),
                        out_offset=None,
                        in_=tabv,
                        in_offset=bass.IndirectOffsetOnAxis(
                            ap=O[:, j * w + c * wc:j * w + (c + 1) * wc],
                            axis=0),
                    )
                    # tracked sem only on each row's last chunk: per-engine
                    # ring FIFO makes it cover the row's earlier chunks, and
                    # the tracked total stays within the counter range.
                    # intermediate chunks get an unwatched sem (walrus
                    # requires sync info on every DGE instruction)
                    if c == n_sub - 1:
                        inst.then_inc(sg, 16)
                    else:
                        inst.then_inc(sj, 16)

    @block.scalar
    def _(ac):
        for t in range(NB):
            if t > 0:
                ac.wait_ge(sd, 16 * (6 + 2 * t) - 16)  # outb shipped (WAR)
            for ch in range(n_ch):
                ac.wait_ge(st, STB * t + 2 * b + 2 * ch + 1)
                ac.activation(hsb[ch % 2][:], hps[ch % 2][:],
                              AFT.Relu).then_inc(sa, 1)
                ac.wait_ge(st, STB * t + 2 * b + 2 * ch + 2)
                ac.activation(outb[:, ch * CH:(ch + 1) * CH], ops[ch % 2][:],
                              AFT.Sigmoid).then_inc(sa, 1)

    for cm in reversed(ctx):
        cm.__exit__(None, None, None)
    return nc


# ---------------- host side ----------------

class _Runner:
    def __init__(self, nc, n_cores):
        import jax
        import numpy as _np
        from jax.sharding import Mesh, PartitionSpec
        from jax.experimental.shard_map import shard_map
        import concourse.mybir as mybir
        from concourse.bass2jax import (
            install_neuronx_cc_hook, _bass_exec_p, partition_id_tensor)
        install_neuronx_cc_hook()
        self.n_cores = n_cores
        pname = nc.partition_id_tensor.name if nc.partition_id_tensor else None
        in_names, out_names, out_avals, zero_outs = [], [], [], []
        for alloc in nc.m.functions[0].allocations:
            if not isinstance(alloc, mybir.MemoryLocationSet):
                continue
            name = alloc.memorylocations[0].name
            if alloc.kind == "ExternalInput":
                if name != pname:
                    in_names.append(name)
            elif alloc.kind == "ExternalOutput":
                shape = tuple(alloc.tensor_shape)
                dtype = mybir.dt.np(alloc.dtype)
                out_names.append(name)
                out_avals.append(jax.core.ShapedArray(shape, dtype))
                zero_outs.append(_np.zeros(shape, dtype))
        self.in_names, self.out_names = in_names, out_names
        self.out_avals, self.zero_outs = out_avals, zero_outs
        n_params, n_outs = len(in_names), len(out_names)
        all_in = in_names + out_names + ([pname] if pname else [])

        def _body(*args):
            operands = list(args)
            if pname is not None:
                operands.append(partition_id_tensor())
            return tuple(_bass_exec_p.bind(
                *operands, out_avals=tuple(out_avals), in_names=tuple(all_in),
                out_names=tuple(out_names), lowering_input_output_aliases=(),
                sim_require_finite=True, sim_require_nnan=True, nc=nc))

        self.n_params, self.n_outs = n_params, n_outs
        donate = tuple(range(n_params, n_params + n_outs))
        devices = jax.devices()[:n_cores]
        mesh = Mesh(_np.asarray(devices), ("core",))
        specs = (PartitionSpec("core"),)
        self.fn = jax.jit(
            shard_map(_body, mesh=mesh, in_specs=specs * (n_params + n_outs),
                      out_specs=specs * n_outs, check_rep=False),
            donate_argnums=donate, keep_unused=True)

    def __call__(self, in_maps):
        import numpy as _np
        n = self.n_cores
        per_core = [[_np.asarray(m[nm]) for nm in self.in_names]
                    for m in in_maps]
        concat_in = [_np.concatenate([per_core[c][i] for c in range(n)], axis=0)
                     for i in range(self.n_params)]
        concat_zeros = [_np.zeros((n * z.shape[0], *z.shape[1:]), z.dtype)
                        for z in self.zero_outs]
        outs = self.fn(*concat_in, *concat_zeros)
        return [
            {nm: _np.asarray(outs[i]).reshape(n, *self.out_avals[i].shape)[c]
             for i, nm in enumerate(self.out_names)}
            for c in range(n)
        ]


_RUNNERS = {}


def _get_runner(NB, b):
    key = (NB, b)
    if key not in _RUNNERS:
        _RUNNERS[key] = _Runner(build_nc(NB, b), N_CORES)
    return _RUNNERS[key]


def _consts(b):
    Lb = L * b
    cfw = np.zeros((128, 11 * Lb), np.float32)
    ciw = np.zeros((128, 7 * Lb), np.int32)
    r1 = np.where(DENSEL, RES + 1, 0)

    def setf(slot, vals):
        cfw[:, slot * Lb:(slot + 1) * Lb] = np.repeat(
            np.asarray(vals, np.float64), b)[None, :]

    def seti(slot, vals):
        ciw[:, slot * Lb:(slot + 1) * Lb] = np.repeat(
            np.asarray(vals, np.int64), b).astype(np.int32)[None, :]

    setf(0, RES)
    setf(1, r1)
    setf(2, r1 * r1)
    for c in range(8):
        dx, dy, dz = (c >> 2) & 1, (c >> 1) & 1, c & 1
        setf(3 + c, np.where(DENSEL,
                             dx + r1 * dy + r1 * r1 * dz + np.arange(L) * T, 0))
    z = np.zeros(L, np.int64)

    def hv(val):
        a = z.copy(); a[N_DENSE:] = val; return a

    seti(0, hv(P2h)); seti(1, hv(P2l)); seti(2, hv(P3h)); seti(3, hv(P3l))
    seti(4, hv(P2p)); seti(5, hv(P3p))
    seti(6, np.arange(L) * T)
    return cfw, ciw


def _prep_core_inputs(points_core, tabflat, w1t, w2t, cfw, ciw, NB, b):
    # pts layout: [128, NB, 3, L, b]; point (t, p, i) coord d replicated /level
    p4 = points_core.reshape(NB, 128, b, 3).transpose(1, 0, 3, 2)  # p t d i
    p5 = np.repeat(p4[:, :, :, None, :], L, axis=3)                # p t d l i
    pts = np.ascontiguousarray(p5, np.float32).reshape(128, NB * 3 * L * b)
    return {"pts": pts, "tab": tabflat, "w1t": w1t, "w2t": w2t,
            "cfw": cfw, "ciw": ciw, "idm": np.eye(128, dtype=np.float32)}


def kernel(points, table, w1, w2):
    points = np.asarray(points, np.float32)
    table = np.asarray(table, np.float32)
    tabflat = np.ascontiguousarray(table.reshape(L * T * F))
    w1t = np.ascontiguousarray(np.asarray(w1, np.float32).T)
    w2t = np.ascontiguousarray(np.asarray(w2, np.float32).T)
    NB, b = N_BATCHES, B_PER_PART
    cfw, ciw = _consts(b)
    runner = _get_runner(NB, b)
    in_maps = [
        _prep_core_inputs(points[c * PTS_PER_CORE:(c + 1) * PTS_PER_CORE],
                          tabflat, w1t, w2t, cfw, ciw, NB, b)
        for c in range(N_CORES)
    ]
    res = runner(in_maps)
    outs = [res[c]["out"].reshape(-1) for c in range(N_CORES)]
    return np.concatenate(outs).reshape(1, 64, 64, 64).astype(np.float32)

